# revision 12
# baseline (speedup 1.0000x reference)
"""Trainium2 Bass kernel for the arc-projection problem (hinge-matmul version).

Full-input contract: kernel(**inputs) takes the unsharded numpy inputs and
returns the full output. Internally shards the batch N=64 across 8 cores
(pure data parallel), runs one SPMD Bass kernel, and gathers.

Algorithm: the reference's searchsorted+lerp projection of arc position s
onto a masked polyline equals a sum of hinge functions.  With
  u_t   = entry + traj_cum_t              (unclipped target arc length)
  cum_j = cumulative masked segment length (cum_0 = 0, cum_255 = total)
  w_j   = sv_j / sl_j masked unit-ish direction, w_{-1} = w_255 = 0
  dwE_j = w_j - w_{j-1}                    (256 hinge coefficients)
the projected point is
  proj_c(t) = base_c + sum_{j=0}^{255} dwE_cj * relu(u_t - cum_j)
exactly (clipping of u to [0, total] is absorbed by the hinge telescope).
That sum is a per-row matmul: stationary R^T[j, t] = relu(tcum_t - (cum_j -
entry)) built by ACT/DVE/Pool from a per-sample broadcast of tcum, moving
operand dwE^T [j, 3], accumulated in PSUM in t-major layout [t, row*3].
The epilogue (distance, per-sample argmin over 32 branches, gather) all
runs in t-major with cross-partition sums done by ones-matmuls on PE.
"""

import sys

import numpy as np

try:
    import concourse.bass as bass
except ImportError:  # pragma: no cover - container without PYTHONPATH set
    sys.path.insert(0, "/opt/trn_rl_repo")
    import concourse.bass as bass

import concourse.tile as tile
from concourse import bacc, masks, mybir
from concourse.bass_utils import run_bass_kernel_spmd

f32 = mybir.dt.float32
bf16 = mybir.dt.bfloat16
AT = mybir.AluOpType
AX = mybir.AxisListType
AF = mybir.ActivationFunctionType

N, T, NB, NP = 64, 128, 16, 256
NCORES = 8
NS = N // NCORES          # samples per core (8)
NB2 = 2 * NB              # fwd + bwd branches (32)
ROWS = NS * NB2           # rows per core (256)
NSEG = NP - 1             # 255
NH = 256                  # hinge count per row: j = 0..255
RT = 128                  # rows per partition-tile
NTILES = ROWS // RT       # 2
SPT = RT // NB2           # samples per row-tile (4)
BIG = 1.0e30


def _view(t, ap_dims, extra_off=0):
    """Strided view of a tile/AP: ap_dims are [step, count] free dims after
    the partition dim (kept from t)."""
    return bass.AP(tensor=t.tensor, offset=t.offset + extra_off,
                   ap=[t.ap[0]] + ap_dims)


def build_nc():
    nc = bacc.Bacc("TRN2", target_bir_lowering=False, debug=False,
                   enable_asserts=False, num_devices=NCORES)

    rp_d = nc.dram_tensor("rp", [ROWS, 3, NP], f32, kind="ExternalInput")
    mk_d = nc.dram_tensor("mk", [ROWS, NP], f32, kind="ExternalInput")
    tj_d = nc.dram_tensor("tj", [NS, 3, T], f32, kind="ExternalInput")
    tjt_d = nc.dram_tensor("tjt", [T, NS * 3], f32, kind="ExternalInput")
    out_d = nc.dram_tensor("out", [T, NS, 3], f32, kind="ExternalOutput")

    dve, act, pool, pe, dma = nc.vector, nc.scalar, nc.gpsimd, nc.tensor, nc.sync

    def _copy(i, out, in_):
        if i % 2 == 0:
            act.copy(out=out, in_=in_)
        else:
            dve.tensor_copy(out=out, in_=in_)

    with tile.TileContext(nc) as tc:
        with (
            tc.tile_pool(name="sing", bufs=1) as sg,
            tc.tile_pool(name="work", bufs=2) as wp,
            tc.tile_pool(name="rel", bufs=2) as rl,
            tc.tile_pool(name="ps", bufs=3, space="PSUM") as pp,
            tc.tile_pool(name="psf", bufs=1, space="PSUM") as pf,
        ):
            # ---------------- static setup ----------------
            ident = sg.tile([128, 128], f32, tag="ident")
            masks.make_identity(nc, ident[:])
            sel = []
            for b in range(SPT):
                selb = sg.tile([128, 128], f32, tag=f"sel{b}", name=f"sel{b}")
                pool.memset(selb, 0.0)
                pool.memset(selb[32 * b:32 * b + 1, :], 1.0)
                sel.append(selb)
            ones_col = sg.tile([128, 1], f32, tag="onesc")
            pool.memset(ones_col, 1.0)
            ones_row = sg.tile([1, 128], f32, tag="onesr")
            pool.memset(ones_row, 1.0)
            sel3 = sg.tile([128, 128], f32, tag="sel3")
            pool.memset(sel3, 0.0)
            for c in range(3):
                pool.memset(sel3[32 * c:32 * c + 1, :], 1.0)
            diag3 = sg.tile([128, ROWS * 3], f32, tag="diag3")
            dve.memset(diag3, 0.0)
            pos_T = sg.tile([128, NS * 3], f32, tag="posT")
            dma.dma_start(out=pos_T, in_=tjt_d.ap())

            # persistent per-tile outputs
            cpT = [[sg.tile([128, RT], f32, tag=f"cpT{k}{h}",
                            name=f"cpT{k}{h}") for h in range(2)]
                    for k in range(NTILES)]
            dwTb = [[sg.tile([128, 18, RT], bf16, tag=f"dwTb{k}{h}",
                             name=f"dwTb{k}{h}") for h in range(2)]
                    for k in range(NTILES)]
            tcumT = [sg.tile([128, RT], f32, tag=f"tcumT{k}",
                             name=f"tcumT{k}") for k in range(NTILES)]
            tcums = [sg.tile([RT, T], f32, tag=f"tcum{k}", name=f"tcum{k}")
                     for k in range(NTILES)]
            tcb = [sg.tile([128, T], f32, tag=f"tcb{s}", name=f"tcb{s}")
                   for s in range(NS)]
            pmb = sg.tile([128, ROWS * 3], f32, tag="pmb")
            diff = sg.tile([128, ROWS * 3], f32, tag="diff")
            dist = sg.tile([128, ROWS], f32, tag="dist")

            # ---------------- pre-pass per row-tile ----------------
            for k in range(NTILES):
                r0 = k * RT
                p = RT
                rpt = wp.tile([p, 3, NP], f32, tag="rpt")
                dma.dma_start(out=rpt, in_=rp_d.ap()[r0:r0 + p])
                mt = wp.tile([p, NP], f32, tag="mt")
                dma.dma_start(out=mt, in_=mk_d.ap()[r0:r0 + p])
                tpb = wp.tile([p, 3, T], f32, tag="tpb")
                dma.dma_start(out=tpb, in_=bass.AP(
                    tensor=tj_d.ap().tensor, offset=k * SPT * 3 * T,
                    ap=[[3 * T, SPT], [0, NB2], [1, 3 * T]]))

                # segment data
                sv = wp.tile([p, 3, NSEG], f32, tag="sv")
                dve.tensor_sub(out=sv, in0=rpt[:, :, 1:NP],
                                in1=rpt[:, :, 0:NSEG])
                sm = wp.tile([p, NSEG], f32, tag="sm")
                dve.tensor_mul(out=sm, in0=mt[:, 1:NP], in1=mt[:, 0:NSEG])
                sq3 = wp.tile([p, 3, NSEG], f32, tag="sq3")
                act.activation(out=sq3, in_=sv, func=AF.Square)
                sl2 = wp.tile([p, NSEG], f32, tag="sl2")
                dve.tensor_add(out=sl2, in0=sq3[:, 0, :], in1=sq3[:, 1, :])
                dve.tensor_add(out=sl2, in0=sl2, in1=sq3[:, 2, :])
                sl2m = wp.tile([p, NSEG], f32, tag="sl2m")
                dve.tensor_mul(out=sl2m, in0=sl2, in1=sm)
                sl = wp.tile([p, NSEG], f32, tag="sl")
                act.activation(out=sl, in_=sl2m, func=AF.Sqrt)

                cum = wp.tile([p, NP], f32, tag="cum")
                zc = wp.tile([p, 1], f32, tag="zc")
                dve.memset(zc, 0.0)
                dve.memset(cum[:, 0:1], 0.0)
                dve.tensor_tensor_scan(
                    out=cum[:, 1:NP], data0=sl, data1=_view(zc, [[0, NSEG]]),
                    initial=0.0, op0=AT.add, op1=AT.add)

                slmax = wp.tile([p, NSEG], f32, tag="slmax")
                dve.tensor_scalar(out=slmax, in0=sl, scalar1=1e-9,
                                   scalar2=None, op0=AT.max)
                rsl = wp.tile([p, NSEG], f32, tag="rsl")
                dve.reciprocal(out=rsl, in_=slmax)
                rslm = wp.tile([p, NSEG], f32, tag="rslm")
                dve.tensor_mul(out=rslm, in0=rsl, in1=sm)
                # wz: w padded with zero columns on both sides
                wz = wp.tile([p, 3, NP + 1], f32, tag="wz")
                pool.memset(_view(wz, [[NP + 1, 3], [1, 1]]), 0.0)
                pool.memset(_view(wz, [[NP + 1, 3], [1, 1]], extra_off=NP), 0.0)
                dve.tensor_mul(out=wz[:, :, 1:NP],
                               in0=sv, in1=_view(rslm, [[0, 3], [1, NSEG]]))
                dwE = wp.tile([p, 3, NH], f32, tag="dwE")
                dve.tensor_sub(out=dwE, in0=wz[:, :, 1:NP + 1],
                                in1=wz[:, :, 0:NP])

                # project p0 on all segments -> entry (one-hot argmin)
                tmp3 = wp.tile([p, 3, NSEG], f32, tag="tmp3")
                for c in range(3):
                    dve.scalar_tensor_tensor(
                        out=tmp3[:, c, :], in0=rpt[:, c, 0:NSEG],
                        scalar=tpb[:, c, 0:1], in1=sv[:, c, :],
                        op0=AT.subtract, op1=AT.mult)
                dotn = wp.tile([p, NSEG], f32, tag="dotn")
                dve.tensor_add(out=dotn, in0=tmp3[:, 0, :], in1=tmp3[:, 1, :])
                dve.tensor_add(out=dotn, in0=dotn, in1=tmp3[:, 2, :])
                svd = wp.tile([p, NSEG], f32, tag="svd")
                dve.tensor_scalar(out=svd, in0=sl2, scalar1=1e-12,
                                   scalar2=None, op0=AT.max)
                rsvd = wp.tile([p, NSEG], f32, tag="rsvd")
                dve.reciprocal(out=rsvd, in_=svd)
                t0 = wp.tile([p, NSEG], f32, tag="t0")
                dve.tensor_mul(out=t0, in0=dotn, in1=rsvd)
                dve.tensor_scalar(out=t0, in0=t0, scalar1=-1.0,
                                   scalar2=0.0, op0=AT.mult, op1=AT.max)
                dve.tensor_scalar(out=t0, in0=t0, scalar1=1.0,
                                   scalar2=None, op0=AT.min)
                s3 = wp.tile([p, 3, NSEG], f32, tag="s3")
                dve.tensor_mul(out=s3, in0=sv,
                                in1=_view(t0, [[0, 3], [1, NSEG]]))
                e3 = wp.tile([p, 3, NSEG], f32, tag="e3")
                for c in range(3):
                    dve.scalar_tensor_tensor(
                        out=e3[:, c, :], in0=rpt[:, c, 0:NSEG],
                        scalar=tpb[:, c, 0:1], in1=s3[:, c, :],
                        op0=AT.subtract, op1=AT.add)
                e3sq = wp.tile([p, 3, NSEG], f32, tag="e3sq")
                act.activation(out=e3sq, in_=e3, func=AF.Square)
                d2 = wp.tile([p, NSEG], f32, tag="d2")
                dve.tensor_add(out=d2, in0=e3sq[:, 0, :], in1=e3sq[:, 1, :])
                dve.tensor_add(out=d2, in0=d2, in1=e3sq[:, 2, :])
                d2m = wp.tile([p, NSEG], f32, tag="d2m")
                dve.tensor_scalar(out=d2m, in0=sm, scalar1=1.0,
                                   scalar2=-BIG, op0=AT.subtract, op1=AT.mult)
                dve.tensor_add(out=d2m, in0=d2m, in1=d2)
                dmin = wp.tile([p, 1], f32, tag="dmin")
                dve.tensor_reduce(out=dmin, in_=d2m, axis=AX.X, op=AT.min)
                ohseg = wp.tile([p, NSEG], f32, tag="ohseg")
                dve.tensor_scalar(out=ohseg, in0=d2m, scalar1=dmin,
                                   scalar2=None, op0=AT.is_equal)
                # keep only the FIRST hot (ties are structural)
                pmax = wp.tile([p, NSEG], f32, tag="pmax")
                dve.tensor_tensor_scan(
                    out=pmax, data0=ohseg, data1=_view(zc, [[0, NSEG]]),
                    initial=0.0, op0=AT.max, op1=AT.add)
                dve.tensor_copy(out=ohseg[:, 0:1], in_=pmax[:, 0:1])
                dve.tensor_sub(out=ohseg[:, 1:NSEG], in0=pmax[:, 1:NSEG],
                               in1=pmax[:, 0:NSEG - 1])
                es = wp.tile([p, NSEG], f32, tag="es")
                dve.tensor_mul(out=es, in0=t0, in1=sl)
                dve.tensor_add(out=es, in0=es, in1=cum[:, 0:NSEG])
                entry = wp.tile([p, 1], f32, tag="entry")
                junk = wp.tile([p, NSEG], f32, tag="junk")
                dve.scalar_tensor_tensor(
                    out=junk, in0=ohseg, scalar=1.0, in1=es,
                    op0=AT.mult, op1=AT.mult, accum_out=entry)

                # base point rp[first valid segment]
                ohf = wp.tile([p, NSEG], f32, tag="ohf")
                dve.tensor_copy(out=ohf[:, 0:1], in_=sm[:, 0:1])
                dve.tensor_sub(out=ohf[:, 1:NSEG], in0=sm[:, 1:NSEG],
                               in1=sm[:, 0:NSEG - 1])
                dve.tensor_scalar(out=ohf, in0=ohf, scalar1=0.0,
                                   scalar2=None, op0=AT.max)
                base3 = wp.tile([p, 3], f32, tag="base3")
                for c in range(3):
                    dve.scalar_tensor_tensor(
                        out=junk, in0=ohf, scalar=1.0, in1=rpt[:, c, 0:NSEG],
                        op0=AT.mult, op1=AT.mult,
                        accum_out=base3[:, c:c + 1])

                # trajectory cumulative arc length
                td = wp.tile([p, 3, T - 1], f32, tag="td")
                dve.tensor_sub(out=td, in0=tpb[:, :, 1:T],
                                in1=tpb[:, :, 0:T - 1])
                td2 = wp.tile([p, 3, T - 1], f32, tag="td2")
                act.activation(out=td2, in_=td, func=AF.Square)
                tl2 = wp.tile([p, T - 1], f32, tag="tl2")
                dve.tensor_add(out=tl2, in0=td2[:, 0, :], in1=td2[:, 1, :])
                dve.tensor_add(out=tl2, in0=tl2, in1=td2[:, 2, :])
                tl = wp.tile([p, T - 1], f32, tag="tl")
                act.activation(out=tl, in_=tl2, func=AF.Sqrt)
                tcum = tcums[k]
                dve.memset(tcum[:, 0:1], 0.0)
                dve.tensor_tensor_scan(
                    out=tcum[:, 1:T], data0=tl, data1=_view(zc, [[0, T - 1]]),
                    initial=0.0, op0=AT.add, op1=AT.add)

                # hinge thresholds in row layout: cum' = cum - entry
                cumpr = wp.tile([p, NP], f32, tag="cumpr")
                dve.tensor_scalar(out=cumpr, in0=cum, scalar1=entry,
                                  scalar2=None, op0=AT.subtract)
                # hinge H coefficients: hcoef = dwE * cum'
                hco = wp.tile([p, 3, NH], f32, tag="hco")
                dve.tensor_mul(out=hco, in0=dwE,
                                in1=_view(cumpr, [[0, 3], [1, NH]]))

                # ---- transposes to j-partition layout (PE) ----
                ci = 0
                for h in range(2):
                    ptr = pp.tile([128, 512], f32, tag="ps")
                    pe.transpose(out=ptr[:, 0:RT],
                                 in_=cumpr[:, h * 128:(h + 1) * 128],
                                 identity=ident)
                    _copy(ci, out=cpT[k][h], in_=ptr[:, 0:RT])
                    ci += 1
                ptr = pp.tile([128, 512], f32, tag="ps")
                pe.transpose(out=ptr[:, 0:RT], in_=tcum, identity=ident)
                _copy(ci, out=tcumT[k], in_=ptr[:, 0:RT])
                ci += 1
                for h in range(2):
                    dw32 = wp.tile([128, 3, RT], f32, tag="dw32")
                    hc32 = wp.tile([128, 3, RT], f32, tag="hc32")
                    for c in range(3):
                        ptr = pp.tile([128, 512], f32, tag="ps")
                        pe.transpose(out=ptr[:, 0:RT],
                                     in_=dwE[:, c, h * 128:(h + 1) * 128],
                                     identity=ident)
                        _copy(ci, out=dw32[:, c, :], in_=ptr[:, 0:RT])
                        ci += 1
                        ptr = pp.tile([128, 512], f32, tag="ps")
                        pe.transpose(out=ptr[:, 0:RT],
                                     in_=hco[:, c, h * 128:(h + 1) * 128],
                                     identity=ident)
                        _copy(ci, out=hc32[:, c, :], in_=ptr[:, 0:RT])
                        ci += 1
                    # triple bf16 split: cols 0-8 = dwE levels, 9-17 = hcoef
                    tgt = dwTb[k][h]
                    for (src32, base) in ((dw32, 0), (hc32, 9)):
                        act.copy(out=tgt[:, base:base + 3, :], in_=src32)
                        r1 = wp.tile([128, 3, RT], f32, tag="rs1")
                        dve.tensor_sub(out=r1, in0=src32,
                                       in1=tgt[:, base:base + 3, :])
                        act.copy(out=tgt[:, base + 3:base + 6, :], in_=r1)
                        r2 = wp.tile([128, 3, RT], f32, tag="rs2")
                        dve.tensor_sub(out=r2, in0=r1,
                                       in1=tgt[:, base + 3:base + 6, :])
                        act.copy(out=tgt[:, base + 6:base + 9, :], in_=r2)
                for c in range(3):
                    ptr = pp.tile([128, 512], f32, tag="ps")
                    pe.transpose(out=ptr[0:1, 0:RT], in_=base3[:, c:c + 1],
                                 identity=ident)
                    _copy(ci, out=_view(diag3[32 * c:32 * c + 1, :],
                                        [[3, RT]], extra_off=k * 384 + c),
                          in_=ptr[0:1, 0:RT])
                    ci += 1

            # ---------------- per-sample tcum broadcasts ----------------
            for s in range(NS):
                k, b = divmod(s, SPT)
                ptc = pp.tile([128, 512], f32, tag="ps")
                pe.matmul(out=ptc[:, 0:T], lhsT=sel[b], rhs=tcums[k],
                          start=True, stop=True)
                act.copy(out=tcb[s], in_=ptc[:, 0:T])

            # ---------------- pmb = pos - base (t-major) ----------------
            for half in range(2):
                pmm = pp.tile([128, 512], f32, tag="ps")
                pe.matmul(out=pmm[:, 0:384], lhsT=sel3,
                          rhs=diag3[:, half * 384:(half + 1) * 384],
                          start=True, stop=True)
                dve.tensor_sub(
                    out=pmb[:, half * 384:(half + 1) * 384],
                    in0=_view(pos_T, [[3, SPT], [0, NB2], [1, 3]],
                              extra_off=half * SPT * 3),
                    in1=pmm[:, 0:384])

            # ---------------- main ge-matmul loop ----------------
            GROUP = 16
            for s in range(NS):
                k = s // SPT
                ges = []
                for h in range(2):
                    ge = rl.tile([128, NB2 * T], bf16, tag=f"GE{h}",
                                 name=f"GE{h}")
                    dve.tensor_tensor(
                        out=ge,
                        in0=_view(tcb[s], [[0, NB2], [1, T]]),
                        in1=_view(cpT[k][h], [[1, NB2], [0, T]],
                                  extra_off=(s % SPT) * NB2),
                        op=AT.is_ge)
                    ges.append(ge)
                for g2 in range(2):
                    fps = pf.tile([128, 512], f32, tag="fb", bufs=2)
                    for rg in range(GROUP):
                        rs = g2 * GROUP + rg
                        r = s * NB2 + rs
                        rr = r % RT
                        pe.matmul(out=fps[:, rg * 18:rg * 18 + 18],
                                  lhsT=_view(ges[0], [[1, T]],
                                             extra_off=rs * T),
                                  rhs=_view(dwTb[k][0], [[RT, 18]],
                                            extra_off=rr),
                                  start=True, stop=False)
                        pe.matmul(out=fps[:, rg * 18:rg * 18 + 18],
                                  lhsT=_view(ges[1], [[1, T]],
                                             extra_off=rs * T),
                                  rhs=_view(dwTb[k][1], [[RT, 18]],
                                            extra_off=rr),
                                  start=False, stop=True)
                    # group epilogue: F = tcum*G - H;  diff = F - pmb
                    r0g = s * NB2 + g2 * GROUP
                    gs = wp.tile([128, GROUP * 3], f32, tag="gs")
                    dve.tensor_copy(out=gs,
                                    in_=_view(fps, [[18, GROUP], [1, 3]]))
                    for off in (3, 6):
                        dve.tensor_add(out=gs, in0=gs,
                                       in1=_view(fps, [[18, GROUP], [1, 3]],
                                                 extra_off=off))
                    hs = wp.tile([128, GROUP * 3], f32, tag="hs")
                    dve.tensor_copy(out=hs,
                                    in_=_view(fps, [[18, GROUP], [1, 3]],
                                              extra_off=9))
                    for off in (12, 15):
                        dve.tensor_add(out=hs, in0=hs,
                                       in1=_view(fps, [[18, GROUP], [1, 3]],
                                                 extra_off=off))
                    dve.tensor_mul(out=gs, in0=gs,
                                   in1=_view(tcumT[k], [[0, GROUP], [0, 3]],
                                             extra_off=(s % SPT) * NB2))
                    dve.tensor_sub(out=gs, in0=gs, in1=hs)
                    dve.tensor_sub(out=diff[:, r0g * 3:r0g * 3 + GROUP * 3],
                                   in0=gs,
                                   in1=pmb[:, r0g * 3:r0g * 3 + GROUP * 3])

            # ---------------- epilogue (t-major) ----------------
            for k in range(NTILES):
                sq = wp.tile([128, 384], f32, tag="sq")
                act.activation(out=sq, in_=diff[:, k * 384:(k + 1) * 384],
                               func=AF.Square)
                d2t = wp.tile([128, RT], f32, tag="d2t")
                dve.tensor_add(out=d2t, in0=_view(sq, [[3, RT]]),
                                in1=_view(sq, [[3, RT]], extra_off=1))
                dve.tensor_add(out=d2t, in0=d2t,
                                in1=_view(sq, [[3, RT]], extra_off=2))
                act.activation(out=dist[:, k * RT:(k + 1) * RT], in_=d2t,
                               func=AF.Sqrt)

            pcost = pp.tile([128, 512], f32, tag="ps")
            for k in range(NTILES):
                pe.matmul(out=pcost[0:1, k * RT:(k + 1) * RT], lhsT=ones_col,
                          rhs=dist[:, k * RT:(k + 1) * RT],
                          start=True, stop=True)
            cost = sg.tile([1, ROWS], f32, tag="cost")
            dve.tensor_copy(out=cost, in_=pcost[0:1, 0:ROWS])
            cmin = sg.tile([1, NS], f32, tag="cmin")
            dve.tensor_reduce(out=cmin, in_=_view(cost, [[NB2, NS], [1, NB2]]),
                              axis=AX.X, op=AT.min)
            oh = sg.tile([1, ROWS], f32, tag="oh")
            dve.tensor_tensor(out=_view(oh, [[NB2, NS], [1, NB2]]),
                              in0=_view(cost, [[NB2, NS], [1, NB2]]),
                              in1=_view(cmin, [[1, NS], [0, NB2]]),
                              op=AT.is_equal)

            outn = sg.tile([128, NS * 3], f32, tag="outn")
            for half in range(2):
                pohb = pp.tile([128, 512], f32, tag="ps")
                pe.matmul(out=pohb[:, 0:384], lhsT=ones_row,
                          rhs=_view(oh, [[1, RT], [0, 3]],
                                    extra_off=half * RT),
                          start=True, stop=True)
                pm = wp.tile([128, 384], f32, tag="pm")
                dve.tensor_mul(out=pm, in0=diff[:, half * 384:(half + 1) * 384],
                               in1=pohb[:, 0:384])
                dve.tensor_reduce(
                    out=outn[:, half * 12:(half + 1) * 12],
                    in_=_view(pm, [[96, SPT], [1, 3], [3, NB2]]),
                    axis=AX.X, op=AT.add)
            final = sg.tile([128, NS * 3], f32, tag="final")
            dve.tensor_add(out=final, in0=outn, in1=pos_T)
            dma.dma_start(out=out_d.ap(), in_=final)

    nc.compile()
    return nc


def marshal_inputs(selected_traj, road_points, road_mask):
    """Host-side layout marshaling (permutations/casts only)."""
    st = np.ascontiguousarray(selected_traj, dtype=np.float32)
    rp = np.ascontiguousarray(road_points, dtype=np.float32)
    rm = np.asarray(road_mask)

    rp_ext = np.concatenate([rp, rp[:, :, ::-1, :]], axis=1)        # [N,NB2,NP,3]
    rp_ext = np.ascontiguousarray(rp_ext.transpose(0, 1, 3, 2))     # [N,NB2,3,NP]
    mk_ext = np.concatenate([rm, rm[:, :, ::-1]], axis=1).astype(np.float32)
    tj = np.ascontiguousarray(st.transpose(0, 2, 1))                # [N,3,T]

    in_maps = []
    for c in range(NCORES):
        s = slice(c * NS, (c + 1) * NS)
        in_maps.append({
            "rp": np.ascontiguousarray(rp_ext[s]).reshape(ROWS, 3, NP),
            "mk": np.ascontiguousarray(mk_ext[s]).reshape(ROWS, NP),
            "tj": np.ascontiguousarray(tj[s]),
            "tjt": np.ascontiguousarray(
                st[s].transpose(1, 0, 2)).reshape(T, NS * 3),
        })
    return in_maps


_NC = None


def kernel(selected_traj, road_points, road_mask):
    global _NC
    if _NC is None:
        _NC = build_nc()
    in_maps = marshal_inputs(selected_traj, road_points, road_mask)
    res = run_bass_kernel_spmd(_NC, in_maps, core_ids=list(range(NCORES)))
    out = np.concatenate(
        [r["out"].transpose(1, 0, 2) for r in res.results], axis=0)
    return np.ascontiguousarray(out.astype(np.float32))


# revision 13
# speedup vs baseline: 1.0409x; 1.0409x over previous
"""Trainium2 Bass kernel for the arc-projection problem (hinge-matmul version).

Full-input contract: kernel(**inputs) takes the unsharded numpy inputs and
returns the full output. Internally shards the batch N=64 across 8 cores
(pure data parallel), runs one SPMD Bass kernel, and gathers.

Algorithm: the reference's searchsorted+lerp projection of arc position s
onto a masked polyline equals a sum of hinge functions.  With
  u_t   = entry + traj_cum_t              (unclipped target arc length)
  cum_j = cumulative masked segment length (cum_0 = 0, cum_255 = total)
  w_j   = sv_j / sl_j masked unit-ish direction, w_{-1} = w_255 = 0
  dwE_j = w_j - w_{j-1}                    (256 hinge coefficients)
the projected point is
  proj_c(t) = base_c + sum_{j=0}^{255} dwE_cj * relu(u_t - cum_j)
exactly (clipping of u to [0, total] is absorbed by the hinge telescope).
That sum is a per-row matmul: stationary R^T[j, t] = relu(tcum_t - (cum_j -
entry)) built by ACT/DVE/Pool from a per-sample broadcast of tcum, moving
operand dwE^T [j, 3], accumulated in PSUM in t-major layout [t, row*3].
The epilogue (distance, per-sample argmin over 32 branches, gather) all
runs in t-major with cross-partition sums done by ones-matmuls on PE.
"""

import sys

import numpy as np

try:
    import concourse.bass as bass
except ImportError:  # pragma: no cover - container without PYTHONPATH set
    sys.path.insert(0, "/opt/trn_rl_repo")
    import concourse.bass as bass

import concourse.tile as tile
from concourse import bacc, masks, mybir
from concourse.bass_utils import run_bass_kernel_spmd

f32 = mybir.dt.float32
bf16 = mybir.dt.bfloat16
AT = mybir.AluOpType
AX = mybir.AxisListType
AF = mybir.ActivationFunctionType

N, T, NB, NP = 64, 128, 16, 256
NCORES = 8
NS = N // NCORES          # samples per core (8)
NB2 = 2 * NB              # fwd + bwd branches (32)
ROWS = NS * NB2           # rows per core (256)
NSEG = NP - 1             # 255
NH = 256                  # hinge count per row: j = 0..255
RT = 128                  # rows per partition-tile
NTILES = ROWS // RT       # 2
SPT = RT // NB2           # samples per row-tile (4)
BIG = 1.0e30


def _view(t, ap_dims, extra_off=0):
    """Strided view of a tile/AP: ap_dims are [step, count] free dims after
    the partition dim (kept from t)."""
    return bass.AP(tensor=t.tensor, offset=t.offset + extra_off,
                   ap=[t.ap[0]] + ap_dims)


def build_nc():
    nc = bacc.Bacc("TRN2", target_bir_lowering=False, debug=False,
                   enable_asserts=False, num_devices=NCORES)

    rp_d = nc.dram_tensor("rp", [ROWS, 3, NP], f32, kind="ExternalInput")
    mk_d = nc.dram_tensor("mk", [ROWS, NP], f32, kind="ExternalInput")
    tj_d = nc.dram_tensor("tj", [NS, 3, T], f32, kind="ExternalInput")
    tjt_d = nc.dram_tensor("tjt", [T, NS * 3], f32, kind="ExternalInput")
    out_d = nc.dram_tensor("out", [T, NS, 3], f32, kind="ExternalOutput")

    dve, act, pool, pe, dma = nc.vector, nc.scalar, nc.gpsimd, nc.tensor, nc.sync

    def _copy(i, out, in_):
        if i % 2 == 0:
            act.copy(out=out, in_=in_)
        else:
            dve.tensor_copy(out=out, in_=in_)

    with tile.TileContext(nc) as tc:
        with (
            tc.tile_pool(name="sing", bufs=1) as sg,
            tc.tile_pool(name="work", bufs=2) as wp,
            tc.tile_pool(name="rel", bufs=2) as rl,
            tc.tile_pool(name="ps", bufs=3, space="PSUM") as pp,
            tc.tile_pool(name="psf", bufs=1, space="PSUM") as pf,
        ):
            # ---------------- static setup ----------------
            ident = sg.tile([128, 128], f32, tag="ident")
            masks.make_identity(nc, ident[:])
            sel = []
            for b in range(SPT):
                selb = sg.tile([128, 128], f32, tag=f"sel{b}", name=f"sel{b}")
                pool.memset(selb, 0.0)
                pool.memset(selb[32 * b:32 * b + 1, :], 1.0)
                sel.append(selb)
            ones_col = sg.tile([128, 1], f32, tag="onesc")
            pool.memset(ones_col, 1.0)
            ones_row = sg.tile([1, 128], f32, tag="onesr")
            pool.memset(ones_row, 1.0)
            sel3 = sg.tile([128, 128], f32, tag="sel3")
            pool.memset(sel3, 0.0)
            for c in range(3):
                pool.memset(sel3[32 * c:32 * c + 1, :], 1.0)
            diag3 = sg.tile([128, ROWS * 3], f32, tag="diag3")
            dve.memset(diag3, 0.0)
            pos_T = sg.tile([128, NS * 3], f32, tag="posT")
            dma.dma_start(out=pos_T, in_=tjt_d.ap())

            # persistent per-tile outputs
            cpT = [[sg.tile([128, RT], f32, tag=f"cpT{k}{h}",
                            name=f"cpT{k}{h}") for h in range(2)]
                    for k in range(NTILES)]
            dwTb = [[sg.tile([128, RT, 18], bf16, tag=f"dwTb{k}{h}",
                             name=f"dwTb{k}{h}") for h in range(2)]
                    for k in range(NTILES)]
            tcumT = [sg.tile([128, RT], f32, tag=f"tcumT{k}",
                             name=f"tcumT{k}") for k in range(NTILES)]
            tcums = [sg.tile([RT, T], f32, tag=f"tcum{k}", name=f"tcum{k}")
                     for k in range(NTILES)]
            tcb = [sg.tile([128, T], f32, tag=f"tcb{s}", name=f"tcb{s}")
                   for s in range(NS)]
            pmb = sg.tile([128, ROWS * 3], f32, tag="pmb")
            diff = sg.tile([128, ROWS * 3], f32, tag="diff")
            dist = sg.tile([128, ROWS], f32, tag="dist")

            # ---------------- pre-pass per row-tile ----------------
            for k in range(NTILES):
                r0 = k * RT
                p = RT
                rpt = wp.tile([p, 3, NP], f32, tag="rpt")
                dma.dma_start(out=rpt, in_=rp_d.ap()[r0:r0 + p])
                mt = wp.tile([p, NP], f32, tag="mt")
                dma.dma_start(out=mt, in_=mk_d.ap()[r0:r0 + p])
                tpb = wp.tile([p, 3, T], f32, tag="tpb")
                dma.dma_start(out=tpb, in_=bass.AP(
                    tensor=tj_d.ap().tensor, offset=k * SPT * 3 * T,
                    ap=[[3 * T, SPT], [0, NB2], [1, 3 * T]]))

                # segment data
                sv = wp.tile([p, 3, NSEG], f32, tag="sv")
                dve.tensor_sub(out=sv, in0=rpt[:, :, 1:NP],
                                in1=rpt[:, :, 0:NSEG])
                sm = wp.tile([p, NSEG], f32, tag="sm")
                dve.tensor_mul(out=sm, in0=mt[:, 1:NP], in1=mt[:, 0:NSEG])
                sq3 = wp.tile([p, 3, NSEG], f32, tag="sq3")
                act.activation(out=sq3, in_=sv, func=AF.Square)
                sl2 = wp.tile([p, NSEG], f32, tag="sl2")
                dve.tensor_add(out=sl2, in0=sq3[:, 0, :], in1=sq3[:, 1, :])
                dve.tensor_add(out=sl2, in0=sl2, in1=sq3[:, 2, :])
                sl2m = wp.tile([p, NSEG], f32, tag="sl2m")
                dve.tensor_mul(out=sl2m, in0=sl2, in1=sm)
                sl = wp.tile([p, NSEG], f32, tag="sl")
                act.activation(out=sl, in_=sl2m, func=AF.Sqrt)

                cum = wp.tile([p, NP], f32, tag="cum")
                zc = wp.tile([p, 1], f32, tag="zc")
                dve.memset(zc, 0.0)
                dve.memset(cum[:, 0:1], 0.0)
                dve.tensor_tensor_scan(
                    out=cum[:, 1:NP], data0=sl, data1=_view(zc, [[0, NSEG]]),
                    initial=0.0, op0=AT.add, op1=AT.add)

                slmax = wp.tile([p, NSEG], f32, tag="slmax")
                dve.tensor_scalar(out=slmax, in0=sl, scalar1=1e-9,
                                   scalar2=None, op0=AT.max)
                rsl = wp.tile([p, NSEG], f32, tag="rsl")
                dve.reciprocal(out=rsl, in_=slmax)
                rslm = wp.tile([p, NSEG], f32, tag="rslm")
                dve.tensor_mul(out=rslm, in0=rsl, in1=sm)
                # wz: w padded with zero columns on both sides
                wz = wp.tile([p, 3, NP + 1], f32, tag="wz")
                pool.memset(_view(wz, [[NP + 1, 3], [1, 1]]), 0.0)
                pool.memset(_view(wz, [[NP + 1, 3], [1, 1]], extra_off=NP), 0.0)
                dve.tensor_mul(out=wz[:, :, 1:NP],
                               in0=sv, in1=_view(rslm, [[0, 3], [1, NSEG]]))
                dwE = wp.tile([p, 3, NH], f32, tag="dwE")
                dve.tensor_sub(out=dwE, in0=wz[:, :, 1:NP + 1],
                                in1=wz[:, :, 0:NP])

                # project p0 on all segments -> entry (one-hot argmin)
                tmp3 = wp.tile([p, 3, NSEG], f32, tag="tmp3")
                for c in range(3):
                    dve.scalar_tensor_tensor(
                        out=tmp3[:, c, :], in0=rpt[:, c, 0:NSEG],
                        scalar=tpb[:, c, 0:1], in1=sv[:, c, :],
                        op0=AT.subtract, op1=AT.mult)
                dotn = wp.tile([p, NSEG], f32, tag="dotn")
                dve.tensor_add(out=dotn, in0=tmp3[:, 0, :], in1=tmp3[:, 1, :])
                dve.tensor_add(out=dotn, in0=dotn, in1=tmp3[:, 2, :])
                svd = wp.tile([p, NSEG], f32, tag="svd")
                dve.tensor_scalar(out=svd, in0=sl2, scalar1=1e-12,
                                   scalar2=None, op0=AT.max)
                rsvd = wp.tile([p, NSEG], f32, tag="rsvd")
                dve.reciprocal(out=rsvd, in_=svd)
                t0 = wp.tile([p, NSEG], f32, tag="t0")
                dve.tensor_mul(out=t0, in0=dotn, in1=rsvd)
                dve.tensor_scalar(out=t0, in0=t0, scalar1=-1.0,
                                   scalar2=0.0, op0=AT.mult, op1=AT.max)
                dve.tensor_scalar(out=t0, in0=t0, scalar1=1.0,
                                   scalar2=None, op0=AT.min)
                s3 = wp.tile([p, 3, NSEG], f32, tag="s3")
                dve.tensor_mul(out=s3, in0=sv,
                                in1=_view(t0, [[0, 3], [1, NSEG]]))
                e3 = wp.tile([p, 3, NSEG], f32, tag="e3")
                for c in range(3):
                    dve.scalar_tensor_tensor(
                        out=e3[:, c, :], in0=rpt[:, c, 0:NSEG],
                        scalar=tpb[:, c, 0:1], in1=s3[:, c, :],
                        op0=AT.subtract, op1=AT.add)
                e3sq = wp.tile([p, 3, NSEG], f32, tag="e3sq")
                act.activation(out=e3sq, in_=e3, func=AF.Square)
                d2 = wp.tile([p, NSEG], f32, tag="d2")
                dve.tensor_add(out=d2, in0=e3sq[:, 0, :], in1=e3sq[:, 1, :])
                dve.tensor_add(out=d2, in0=d2, in1=e3sq[:, 2, :])
                d2m = wp.tile([p, NSEG], f32, tag="d2m")
                dve.tensor_scalar(out=d2m, in0=sm, scalar1=1.0,
                                   scalar2=-BIG, op0=AT.subtract, op1=AT.mult)
                dve.tensor_add(out=d2m, in0=d2m, in1=d2)
                dmin = wp.tile([p, 1], f32, tag="dmin")
                dve.tensor_reduce(out=dmin, in_=d2m, axis=AX.X, op=AT.min)
                ohseg = wp.tile([p, NSEG], f32, tag="ohseg")
                dve.tensor_scalar(out=ohseg, in0=d2m, scalar1=dmin,
                                   scalar2=None, op0=AT.is_equal)
                # keep only the FIRST hot (ties are structural)
                pmax = wp.tile([p, NSEG], f32, tag="pmax")
                dve.tensor_tensor_scan(
                    out=pmax, data0=ohseg, data1=_view(zc, [[0, NSEG]]),
                    initial=0.0, op0=AT.max, op1=AT.add)
                dve.tensor_copy(out=ohseg[:, 0:1], in_=pmax[:, 0:1])
                dve.tensor_sub(out=ohseg[:, 1:NSEG], in0=pmax[:, 1:NSEG],
                               in1=pmax[:, 0:NSEG - 1])
                es = wp.tile([p, NSEG], f32, tag="es")
                dve.tensor_mul(out=es, in0=t0, in1=sl)
                dve.tensor_add(out=es, in0=es, in1=cum[:, 0:NSEG])
                entry = wp.tile([p, 1], f32, tag="entry")
                junk = wp.tile([p, NSEG], f32, tag="junk")
                dve.scalar_tensor_tensor(
                    out=junk, in0=ohseg, scalar=1.0, in1=es,
                    op0=AT.mult, op1=AT.mult, accum_out=entry)

                # base point rp[first valid segment]
                ohf = wp.tile([p, NSEG], f32, tag="ohf")
                dve.tensor_copy(out=ohf[:, 0:1], in_=sm[:, 0:1])
                dve.tensor_sub(out=ohf[:, 1:NSEG], in0=sm[:, 1:NSEG],
                               in1=sm[:, 0:NSEG - 1])
                dve.tensor_scalar(out=ohf, in0=ohf, scalar1=0.0,
                                   scalar2=None, op0=AT.max)
                base3 = wp.tile([p, 3], f32, tag="base3")
                for c in range(3):
                    dve.scalar_tensor_tensor(
                        out=junk, in0=ohf, scalar=1.0, in1=rpt[:, c, 0:NSEG],
                        op0=AT.mult, op1=AT.mult,
                        accum_out=base3[:, c:c + 1])

                # trajectory cumulative arc length
                td = wp.tile([p, 3, T - 1], f32, tag="td")
                dve.tensor_sub(out=td, in0=tpb[:, :, 1:T],
                                in1=tpb[:, :, 0:T - 1])
                td2 = wp.tile([p, 3, T - 1], f32, tag="td2")
                act.activation(out=td2, in_=td, func=AF.Square)
                tl2 = wp.tile([p, T - 1], f32, tag="tl2")
                dve.tensor_add(out=tl2, in0=td2[:, 0, :], in1=td2[:, 1, :])
                dve.tensor_add(out=tl2, in0=tl2, in1=td2[:, 2, :])
                tl = wp.tile([p, T - 1], f32, tag="tl")
                act.activation(out=tl, in_=tl2, func=AF.Sqrt)
                tcum = tcums[k]
                dve.memset(tcum[:, 0:1], 0.0)
                dve.tensor_tensor_scan(
                    out=tcum[:, 1:T], data0=tl, data1=_view(zc, [[0, T - 1]]),
                    initial=0.0, op0=AT.add, op1=AT.add)

                # hinge thresholds in row layout: cum' = cum - entry
                cumpr = wp.tile([p, NP], f32, tag="cumpr")
                dve.tensor_scalar(out=cumpr, in0=cum, scalar1=entry,
                                  scalar2=None, op0=AT.subtract)
                # hinge H coefficients: hcoef = dwE * cum'
                hco = wp.tile([p, 3, NH], f32, tag="hco")
                dve.tensor_mul(out=hco, in0=dwE,
                                in1=_view(cumpr, [[0, 3], [1, NH]]))

                # ---- transposes to j-partition layout (PE) ----
                ci = 0
                for h in range(2):
                    ptr = pp.tile([128, 512], f32, tag="ps")
                    pe.transpose(out=ptr[:, 0:RT],
                                 in_=cumpr[:, h * 128:(h + 1) * 128],
                                 identity=ident)
                    _copy(ci, out=cpT[k][h], in_=ptr[:, 0:RT])
                    ci += 1
                ptr = pp.tile([128, 512], f32, tag="ps")
                pe.transpose(out=ptr[:, 0:RT], in_=tcum, identity=ident)
                _copy(ci, out=tcumT[k], in_=ptr[:, 0:RT])
                ci += 1
                for h in range(2):
                    dw32 = wp.tile([128, 3, RT], f32, tag="dw32")
                    hc32 = wp.tile([128, 3, RT], f32, tag="hc32")
                    for c in range(3):
                        ptr = pp.tile([128, 512], f32, tag="ps")
                        pe.transpose(out=ptr[:, 0:RT],
                                     in_=dwE[:, c, h * 128:(h + 1) * 128],
                                     identity=ident)
                        _copy(ci, out=dw32[:, c, :], in_=ptr[:, 0:RT])
                        ci += 1
                        ptr = pp.tile([128, 512], f32, tag="ps")
                        pe.transpose(out=ptr[:, 0:RT],
                                     in_=hco[:, c, h * 128:(h + 1) * 128],
                                     identity=ident)
                        _copy(ci, out=hc32[:, c, :], in_=ptr[:, 0:RT])
                        ci += 1
                    # triple bf16 split: cols 0-8 = dwE levels, 9-17 = hcoef
                    tgt = dwTb[k][h]

                    def tv(base):
                        return _view(tgt, [[1, 3], [18, RT]], extra_off=base)

                    for (src32, base) in ((dw32, 0), (hc32, 9)):
                        s3v = _view(src32, [[RT, 3], [1, RT]])
                        act.copy(out=tv(base), in_=s3v)
                        r1 = wp.tile([128, 3, RT], f32, tag="rs1")
                        dve.tensor_sub(out=_view(r1, [[RT, 3], [1, RT]]),
                                       in0=s3v, in1=tv(base))
                        act.copy(out=tv(base + 3),
                                 in_=_view(r1, [[RT, 3], [1, RT]]))
                        r2 = wp.tile([128, 3, RT], f32, tag="rs2")
                        dve.tensor_sub(out=_view(r2, [[RT, 3], [1, RT]]),
                                       in0=_view(r1, [[RT, 3], [1, RT]]),
                                       in1=tv(base + 3))
                        act.copy(out=tv(base + 6),
                                 in_=_view(r2, [[RT, 3], [1, RT]]))
                for c in range(3):
                    ptr = pp.tile([128, 512], f32, tag="ps")
                    pe.transpose(out=ptr[0:1, 0:RT], in_=base3[:, c:c + 1],
                                 identity=ident)
                    _copy(ci, out=_view(diag3[32 * c:32 * c + 1, :],
                                        [[3, RT]], extra_off=k * 384 + c),
                          in_=ptr[0:1, 0:RT])
                    ci += 1

            # ---------------- per-sample tcum broadcasts ----------------
            for s in range(NS):
                k, b = divmod(s, SPT)
                ptc = pp.tile([128, 512], f32, tag="ps")
                pe.matmul(out=ptc[:, 0:T], lhsT=sel[b], rhs=tcums[k],
                          start=True, stop=True)
                act.copy(out=tcb[s], in_=ptc[:, 0:T])

            # ---------------- pmb = pos - base (t-major) ----------------
            for half in range(2):
                pmm = pp.tile([128, 512], f32, tag="ps")
                pe.matmul(out=pmm[:, 0:384], lhsT=sel3,
                          rhs=diag3[:, half * 384:(half + 1) * 384],
                          start=True, stop=True)
                dve.tensor_sub(
                    out=pmb[:, half * 384:(half + 1) * 384],
                    in0=_view(pos_T, [[3, SPT], [0, NB2], [1, 3]],
                              extra_off=half * SPT * 3),
                    in1=pmm[:, 0:384])

            # ---------------- main ge-matmul loop ----------------
            GROUP = 16
            for s in range(NS):
                k = s // SPT
                ges = []
                for h in range(2):
                    ge = rl.tile([128, NB2 * T], bf16, tag=f"GE{h}",
                                 name=f"GE{h}")
                    dve.tensor_tensor(
                        out=ge,
                        in0=_view(tcb[s], [[0, NB2], [1, T]]),
                        in1=_view(cpT[k][h], [[1, NB2], [0, T]],
                                  extra_off=(s % SPT) * NB2),
                        op=AT.is_ge)
                    ges.append(ge)
                for g2 in range(2):
                    fps = pf.tile([128, 512], f32, tag="fb", bufs=2)
                    for rg in range(GROUP):
                        rs = g2 * GROUP + rg
                        r = s * NB2 + rs
                        rr = r % RT
                        pe.matmul(out=fps[:, rg * 18:rg * 18 + 18],
                                  lhsT=_view(ges[0], [[1, T]],
                                             extra_off=rs * T),
                                  rhs=_view(dwTb[k][0], [[1, 18]],
                                            extra_off=rr * 18),
                                  start=True, stop=False)
                        pe.matmul(out=fps[:, rg * 18:rg * 18 + 18],
                                  lhsT=_view(ges[1], [[1, T]],
                                             extra_off=rs * T),
                                  rhs=_view(dwTb[k][1], [[1, 18]],
                                            extra_off=rr * 18),
                                  start=False, stop=True)
                    # group epilogue: F = tcum*G - H;  diff = F - pmb
                    r0g = s * NB2 + g2 * GROUP
                    gs = wp.tile([128, GROUP * 3], f32, tag="gs")
                    dve.tensor_copy(out=gs,
                                    in_=_view(fps, [[18, GROUP], [1, 3]]))
                    for off in (3, 6):
                        dve.tensor_add(out=gs, in0=gs,
                                       in1=_view(fps, [[18, GROUP], [1, 3]],
                                                 extra_off=off))
                    hs = wp.tile([128, GROUP * 3], f32, tag="hs")
                    dve.tensor_copy(out=hs,
                                    in_=_view(fps, [[18, GROUP], [1, 3]],
                                              extra_off=9))
                    for off in (12, 15):
                        dve.tensor_add(out=hs, in0=hs,
                                       in1=_view(fps, [[18, GROUP], [1, 3]],
                                                 extra_off=off))
                    dve.tensor_mul(out=gs, in0=gs,
                                   in1=_view(tcumT[k], [[0, GROUP], [0, 3]],
                                             extra_off=(s % SPT) * NB2))
                    dve.tensor_sub(out=gs, in0=gs, in1=hs)
                    dve.tensor_sub(out=diff[:, r0g * 3:r0g * 3 + GROUP * 3],
                                   in0=gs,
                                   in1=pmb[:, r0g * 3:r0g * 3 + GROUP * 3])

            # ---------------- epilogue (t-major) ----------------
            for k in range(NTILES):
                sq = wp.tile([128, 384], f32, tag="sq")
                act.activation(out=sq, in_=diff[:, k * 384:(k + 1) * 384],
                               func=AF.Square)
                d2t = wp.tile([128, RT], f32, tag="d2t")
                dve.tensor_add(out=d2t, in0=_view(sq, [[3, RT]]),
                                in1=_view(sq, [[3, RT]], extra_off=1))
                dve.tensor_add(out=d2t, in0=d2t,
                                in1=_view(sq, [[3, RT]], extra_off=2))
                act.activation(out=dist[:, k * RT:(k + 1) * RT], in_=d2t,
                               func=AF.Sqrt)

            pcost = pp.tile([128, 512], f32, tag="ps")
            for k in range(NTILES):
                pe.matmul(out=pcost[0:1, k * RT:(k + 1) * RT], lhsT=ones_col,
                          rhs=dist[:, k * RT:(k + 1) * RT],
                          start=True, stop=True)
            cost = sg.tile([1, ROWS], f32, tag="cost")
            dve.tensor_copy(out=cost, in_=pcost[0:1, 0:ROWS])
            cmin = sg.tile([1, NS], f32, tag="cmin")
            dve.tensor_reduce(out=cmin, in_=_view(cost, [[NB2, NS], [1, NB2]]),
                              axis=AX.X, op=AT.min)
            oh = sg.tile([1, ROWS], f32, tag="oh")
            dve.tensor_tensor(out=_view(oh, [[NB2, NS], [1, NB2]]),
                              in0=_view(cost, [[NB2, NS], [1, NB2]]),
                              in1=_view(cmin, [[1, NS], [0, NB2]]),
                              op=AT.is_equal)

            outn = sg.tile([128, NS * 3], f32, tag="outn")
            for half in range(2):
                pohb = pp.tile([128, 512], f32, tag="ps")
                pe.matmul(out=pohb[:, 0:384], lhsT=ones_row,
                          rhs=_view(oh, [[1, RT], [0, 3]],
                                    extra_off=half * RT),
                          start=True, stop=True)
                pm = wp.tile([128, 384], f32, tag="pm")
                dve.tensor_mul(out=pm, in0=diff[:, half * 384:(half + 1) * 384],
                               in1=pohb[:, 0:384])
                dve.tensor_reduce(
                    out=outn[:, half * 12:(half + 1) * 12],
                    in_=_view(pm, [[96, SPT], [1, 3], [3, NB2]]),
                    axis=AX.X, op=AT.add)
            final = sg.tile([128, NS * 3], f32, tag="final")
            dve.tensor_add(out=final, in0=outn, in1=pos_T)
            dma.dma_start(out=out_d.ap(), in_=final)

    nc.compile()
    return nc


def marshal_inputs(selected_traj, road_points, road_mask):
    """Host-side layout marshaling (permutations/casts only)."""
    st = np.ascontiguousarray(selected_traj, dtype=np.float32)
    rp = np.ascontiguousarray(road_points, dtype=np.float32)
    rm = np.asarray(road_mask)

    rp_ext = np.concatenate([rp, rp[:, :, ::-1, :]], axis=1)        # [N,NB2,NP,3]
    rp_ext = np.ascontiguousarray(rp_ext.transpose(0, 1, 3, 2))     # [N,NB2,3,NP]
    mk_ext = np.concatenate([rm, rm[:, :, ::-1]], axis=1).astype(np.float32)
    tj = np.ascontiguousarray(st.transpose(0, 2, 1))                # [N,3,T]

    in_maps = []
    for c in range(NCORES):
        s = slice(c * NS, (c + 1) * NS)
        in_maps.append({
            "rp": np.ascontiguousarray(rp_ext[s]).reshape(ROWS, 3, NP),
            "mk": np.ascontiguousarray(mk_ext[s]).reshape(ROWS, NP),
            "tj": np.ascontiguousarray(tj[s]),
            "tjt": np.ascontiguousarray(
                st[s].transpose(1, 0, 2)).reshape(T, NS * 3),
        })
    return in_maps


_NC = None


def kernel(selected_traj, road_points, road_mask):
    global _NC
    if _NC is None:
        _NC = build_nc()
    in_maps = marshal_inputs(selected_traj, road_points, road_mask)
    res = run_bass_kernel_spmd(_NC, in_maps, core_ids=list(range(NCORES)))
    out = np.concatenate(
        [r["out"].transpose(1, 0, 2) for r in res.results], axis=0)
    return np.ascontiguousarray(out.astype(np.float32))
